# revision 17
# baseline (speedup 1.0000x reference)
"""Trainium2 Bass kernel for nn_EstimateGrassmann — prefix-tree version.

Math: p_b = det(sigma - diag(z_b)) with z = 1-x; only the DIAGONAL differs
across samples, so samples sharing the first K bits of z share the first K
Gaussian-elimination steps exactly.  We build a per-core table of all 4096
local prefix Schur complements (trailing 17x17 matrices + running sum of
log(pivot^2)), then each sample gathers its entry via indirect DMA and
finishes the last 17 pivots.

Tree construction per core (samples are routed to cores by the top 3 bits
of the 15-bit prefix, so each core only builds its own 4096-node subtree):
  - bootstrap: each of the 128 partitions eliminates steps 0..9 for its own
    10-bit prefix (3 core bits + 7 partition bits), z supplied as input.
  - levels 10..14: partition-local doubling in the free dim; child slot
    2s+z of parent slot s, so no cross-partition traffic.
  - the level-15 table ([P, 32*290]) goes to DRAM; 68 per-tile indirect
    DMAs gather per-sample rows back into [P, 17*290] group tiles.

Host side: sort samples by prefix, pad each core's shard to 68*128 = 8704
(real max ~8316 for Binomial(65536, 1/8)), mask pad slots out of the mean.
All per-core inputs are packed into ONE int32 [9112, 32] blob — per-call
overhead on the axon/PJRT path scales with the number of input buffers
(~1 ms each), dwarfing the ~0.5 ms device time otherwise.
"""

import numpy as np

DIM = 32
BATCH = 65536
NCORES = 8
P = 128
KPRE = 15                  # prefix bits shared via the tree
BOOT = 10                  # bootstrap levels (3 core bits + 7 partition bits)
SLOTS = 1 << (KPRE - BOOT) # 32 table slots per partition
D2 = DIM - KPRE            # 17: trailing matrix size for the sample phase
ROW = D2 * D2 + 1          # 290 floats per table row (matrix + log-partial)
NNODES = P * SLOTS         # 4096 table rows per core
NTILES = 68                # sample capacity per core = 68*128 = 8704
CAP = NTILES * P
GT = 17                    # tiles per elimination group
NGROUPS = NTILES // GT     # 4

# packed input blob layout (rows of 32 int32 words)
ZF_ROWS = (P * NTILES * D2) // DIM  # zf [128, 68*17] f32 -> 4624 rows
ROW_B = ZF_ROWS            # B  [32, 32] f32 (bitcast)
ROW_C = ZF_ROWS + 32       # C  [32, 32] f32
ROW_EYE = ZF_ROWS + 64     # eye [32, 32] f32
ROW_ZB = ZF_ROWS + 96      # zb [128, 10] f32 -> 40 rows
ROW_IDX = ZF_ROWS + 136    # idx [128, 68] i32 -> 272 rows
BLOB_ROWS = ZF_ROWS + 136 + (P * NTILES) // DIM   # 5032

ALPHA_C = 1.0 / 4.0
ALPHA_LAM = 1.0 / 5.0
NEWTON_ITERS = 7

_cache = {}


def _build():
    import concourse.bass as bass
    import concourse.mybir as mybir
    from concourse.tile import TileContext

    fp32 = mybir.dt.float32
    i32 = mybir.dt.int32
    AF = mybir.ActivationFunctionType
    OP = mybir.AluOpType
    AX = mybir.AxisListType

    nc = bass.Bass()
    blob_d = nc.dram_tensor("blob", [BLOB_ROWS, DIM], i32, kind="ExternalInput")
    out_d = nc.dram_tensor("out", [P, NTILES], fp32, kind="ExternalOutput")

    zf_src = blob_d[0:ZF_ROWS, :].bitcast(fp32) \
        .rearrange("a b -> (a b)").rearrange("(p f) -> p f", f=NTILES * D2)
    b_src = blob_d[ROW_B:ROW_B + 32, :].bitcast(fp32)
    c_src = blob_d[ROW_C:ROW_C + 32, :].bitcast(fp32)
    eye_src = blob_d[ROW_EYE:ROW_EYE + 32, :].bitcast(fp32)
    zb_src = blob_d[ROW_ZB:ROW_ZB + 40, :].bitcast(fp32) \
        .rearrange("a b -> (a b)").rearrange("(p k) -> p k", k=BOOT)
    idx_src = blob_d[ROW_IDX:ROW_IDX + 272, :] \
        .rearrange("a b -> (a b)").rearrange("(p k) -> p k", k=NTILES)

    with TileContext(nc) as tc:
        with tc.tile_pool(name="const", bufs=1) as cpool, \
             tc.tile_pool(name="setup", bufs=1) as spool, \
             tc.tile_pool(name="psum", bufs=2, space="PSUM") as qpool, \
             tc.tile_pool(name="dram", bufs=1, space="DRAM") as dpool, \
             tc.tile_pool(name="tabdram", bufs=1, space="DRAM") as tdpool, \
             tc.tile_pool(name="tree", bufs=1) as tpool, \
             tc.tile_pool(name="big", bufs=1) as gpool, \
             tc.tile_pool(name="r", bufs=4) as rpool, \
             tc.tile_pool(name="d2", bufs=2) as d2pool:

            eye = cpool.tile([DIM, DIM], fp32, name="eye_sb")
            ome = cpool.tile([DIM, DIM], fp32, name="ome_sb")
            eye2 = cpool.tile([DIM, DIM], fp32, name="eye2_sb")
            nc.sync.dma_start(eye[:], eye_src)
            nc.vector.tensor_scalar(ome[:], eye[:], -1.0, 1.0,
                                    op0=OP.mult, op1=OP.add)
            nc.vector.tensor_scalar(eye2[:], eye[:], 2.0, None, op0=OP.mult)

            def stabilize(m_sb, nm):
                off = spool.tile([DIM, DIM], fp32, name=f"off_{nm}")
                rl = spool.tile([DIM, DIM], fp32, name=f"rl_{nm}")
                ab = spool.tile([DIM, DIM], fp32, name=f"ab_{nm}")
                ab2 = spool.tile([DIM, DIM], fp32, name=f"ab2_{nm}")
                rs = spool.tile([DIM, 1], fp32, name=f"rs_{nm}")
                rs2 = spool.tile([DIM, 1], fp32, name=f"rs2_{nm}")
                st = spool.tile([DIM, DIM], fp32, name=f"st_{nm}")
                nc.vector.tensor_mul(off[:], m_sb[:], ome[:])
                nc.scalar.activation(rl[:], m_sb[:], AF.Relu)
                nc.scalar.activation(ab[:], off[:], AF.Abs)
                nc.vector.tensor_mul(ab2[:], rl[:], eye[:])
                nc.vector.tensor_reduce(rs[:], ab[:], axis=AX.X, op=OP.add)
                nc.vector.tensor_reduce(rs2[:], ab2[:], axis=AX.X, op=OP.add)
                nc.vector.tensor_add(rs[:], rs[:], rs2[:])
                nc.vector.scalar_tensor_tensor(
                    st[:], eye[:], rs[:, 0:1], off[:], op0=OP.mult, op1=OP.add)
                return st

            def transpose32(src_sb, nm):
                ps = qpool.tile([DIM, DIM], fp32, name=f"pt_{nm}", tag="ps")
                dst = spool.tile([DIM, DIM], fp32, name=f"tr_{nm}")
                nc.tensor.transpose(ps[:], src_sb[:], eye[:])
                nc.scalar.copy(dst[:], ps[:])
                return dst

            def inv32(a_sb, alpha, nm):
                at = transpose32(a_sb, nm)
                xx = spool.tile([DIM, DIM], fp32, name=f"x_{nm}")
                xt = spool.tile([DIM, DIM], fp32, name=f"xt_{nm}")
                nc.vector.tensor_scalar(xx[:], eye[:], alpha, None, op0=OP.mult)
                nc.vector.tensor_scalar(xt[:], eye[:], alpha, None, op0=OP.mult)
                for it in range(NEWTON_ITERS):
                    t1 = qpool.tile([DIM, DIM], fp32, name=f"nt_{nm}_{it}", tag="ps")
                    nc.tensor.matmul(t1[:], at[:], xx[:], start=True, stop=True)
                    w = spool.tile([DIM, DIM], fp32, name=f"w_{nm}_{it}", tag="w")
                    nc.vector.scalar_tensor_tensor(
                        w[:], t1[:], -1.0, eye2[:], op0=OP.mult, op1=OP.add)
                    x2 = qpool.tile([DIM, DIM], fp32, name=f"x2_{nm}_{it}", tag="ps2")
                    xt2 = qpool.tile([DIM, DIM], fp32, name=f"xt2_{nm}_{it}", tag="ps3")
                    nc.tensor.matmul(x2[:], xt[:], w[:], start=True, stop=True)
                    nc.tensor.matmul(xt2[:], w[:], xt[:], start=True, stop=True)
                    nc.vector.tensor_copy(xx[:], x2[:])
                    nc.scalar.copy(xt[:], xt2[:])
                return xx, xt

            # ---- phase A: sigma = inv(stab(B) @ inv(stab(C)) + I) ----
            b_sb = spool.tile([DIM, DIM], fp32, name="b_sb")
            c_sb = spool.tile([DIM, DIM], fp32, name="c_sb")
            nc.sync.dma_start(b_sb[:], b_src)
            nc.sync.dma_start(c_sb[:], c_src)
            bs = stabilize(b_sb, "b")
            cs_ = stabilize(c_sb, "c")
            invc, _ = inv32(cs_, ALPHA_C, "c")
            bt = transpose32(bs, "bt")
            lamp = qpool.tile([DIM, DIM], fp32, name="lamp", tag="ps")
            nc.tensor.matmul(lamp[:], bt[:], invc[:], start=True, stop=True)
            lam = spool.tile([DIM, DIM], fp32, name="lam")
            nc.vector.tensor_add(lam[:], lamp[:], eye[:])
            sigma, _ = inv32(lam, ALPHA_LAM, "s")

            sig_dram = dpool.tile([DIM, DIM], fp32, name="sig_dram")
            nc.sync.dma_start(sig_dram[:], sigma[:])
            sig_rep = tpool.tile([P, DIM * DIM], fp32, name="sig_rep")
            src = sig_dram[:].rearrange("a b -> (a b)").unsqueeze(0) \
                             .broadcast_to([P, DIM * DIM])
            nc.sync.dma_start(sig_rep[:], src)

            # ---- phase B: bootstrap (steps 0..9, one prefix path/partition)
            zb = spool.tile([P, BOOT], fp32, name="zb_sb")
            nc.sync.dma_start(zb[:], zb_src)
            T = tpool.tile([P, DIM * DIM], fp32, name="T_boot")
            nc.vector.tensor_copy(T[:], sig_rep[:])
            Tm = T.rearrange("p (i j) -> p i j", j=DIM)
            Pv = spool.tile([P, BOOT], fp32, name="Pv_boot")
            for k in range(BOOT):
                n = DIM - 1 - k
                nc.vector.tensor_sub(Pv[:, k:k + 1],
                                     T[:, 33 * k:33 * k + 1], zb[:, k:k + 1])
                rv = rpool.tile([P, 1], fp32, name=f"rvb_{k}", tag="rv")
                nc.vector.reciprocal(rv[:], Pv[:, k:k + 1])
                csb_ = rpool.tile([P, n], fp32, name=f"csb_{k}", tag="cs")
                nc.vector.tensor_scalar(csb_[:], Tm[:, k + 1:, k], rv[:, 0:1],
                                        None, op0=OP.mult)
                tvb = d2pool.tile([P, GT * (D2 - 1) * (D2 - 1)], fp32,
                                  name=f"tvb_{k}", tag="tv0")[:, :n * n]
                tvv = tvb.rearrange("p (i j) -> p i j", j=n)
                nc.vector.tensor_mul(
                    tvv,
                    csb_[:].unsqueeze(2).broadcast_to([P, n, n]),
                    Tm[:, k:k + 1, k + 1:].broadcast_to([P, n, n]))
                nc.vector.tensor_sub(Tm[:, k + 1:, k + 1:],
                                     Tm[:, k + 1:, k + 1:], tvv)
            d2b = spool.tile([P, BOOT], fp32, name="d2_boot")
            nc.scalar.activation(d2b[:], Pv[:], AF.Square)
            lnb = spool.tile([P, BOOT], fp32, name="ln_boot")
            nc.scalar.activation(lnb[:], d2b[:], AF.Ln)
            bp = spool.tile([P, 1], fp32, name="bp_boot")
            nc.vector.tensor_reduce(bp[:], lnb[:], axis=AX.X, op=OP.add)

            import os as _os2
            _ab = bool(_os2.environ.get("V2_PHASE_AB"))

            # ---- phase C: doubling levels 10..14 ----
            n10 = DIM - BOOT                      # 22
            tabX = tpool.tile([P, 16 * 18 * 18], fp32, name="tabX")
            tabY = tpool.tile([P, SLOTS * ROW], fp32, name="tabY")
            if not _ab:
                nc.vector.tensor_copy(
                    tabX[:, :n10 * n10].rearrange("p (i j) -> p i j", j=n10),
                    Tm[:, BOOT:, BOOT:])
                par = spool.tile([P, 1], fp32, name="par10")
                nc.vector.tensor_copy(par[:], bp[:])

            for k in ([] if _ab else range(BOOT, KPRE)):
                S = 1 << (k - BOOT)
                n = DIM - k
                n2 = n - 1
                last = (k == KPRE - 1)
                stride = ROW if last else n2 * n2
                src_buf = tabX if (k - BOOT) % 2 == 0 else tabY
                dst_buf = tabY if (k - BOOT) % 2 == 0 else tabX
                tab = src_buf[:, :S * n * n]
                tabB = dst_buf[:, :2 * S * stride]
                TmA = tab.rearrange("p (s i j) -> p s i j", i=n, j=n)
                TmB = tabB.rearrange("p (s f) -> p s f", f=stride)
                parB = spool.tile([P, 2 * S], fp32, name=f"par{k + 1}") \
                    if not last else None
                PvL = spool.tile([P, 2 * S], fp32, name=f"PvL{k}")
                nc.vector.tensor_copy(PvL[:, 0::2], TmA[:, :, 0, 0])
                nc.vector.tensor_scalar(PvL[:, 1::2], TmA[:, :, 0, 0], -1.0,
                                        None, op0=OP.add)
                rv = rpool.tile([P, 2 * S], fp32, name=f"rvl_{k}", tag="rv")
                nc.vector.reciprocal(rv[:], PvL[:])
                d2l = d2pool.tile([P, 2 * S], fp32, name=f"d2l_{k}", tag="d2")
                nc.scalar.activation(d2l[:], PvL[:], AF.Square)
                lnl = d2pool.tile([P, 2 * S], fp32, name=f"lnl_{k}", tag="ln")
                nc.scalar.activation(lnl[:], d2l[:], AF.Ln)
                for z in (0, 1):
                    pdst = TmB[:, z::2, n2 * n2] if last else parB[:, z::2]
                    nc.vector.tensor_add(pdst, par[:], lnl[:, z::2])
                    csl = rpool.tile([P, S * n2], fp32, name=f"csl_{k}_{z}",
                                     tag="cs")
                    cs3 = csl.rearrange("p (s i) -> p s i", i=n2)
                    nc.vector.tensor_mul(
                        cs3, TmA[:, :, 1:, 0],
                        rv[:, z::2].unsqueeze(2).broadcast_to([P, S, n2]))
                    tvl = d2pool.tile([P, S * n2 * n2], fp32,
                                      name=f"tvl_{k}_{z}", tag="tv0")
                    tv4 = tvl.rearrange("p (s i j) -> p s i j", i=n2, j=n2)
                    nc.vector.tensor_mul(
                        tv4,
                        cs3[:, :, :].unsqueeze(3).broadcast_to([P, S, n2, n2]),
                        TmA[:, :, 0:1, 1:].broadcast_to([P, S, n2, n2]))
                    child = TmB[:, z::2, 0:n2 * n2] \
                        .rearrange("p s (i j) -> p s i j", j=n2)
                    nc.vector.tensor_sub(child, TmA[:, :, 1:, 1:], tv4)
                par = parB

            # ---- phase D: table to DRAM ----
            table = tdpool.tile([NNODES, ROW], fp32, name="table_dram")
            if not _ab:
                nc.sync.dma_start(
                    table[:].rearrange("(p s) f -> p (s f)", p=P),
                    tabY[:, :SLOTS * ROW])

            # ---- phase E: gather + per-sample elimination ----
            idx = spool.tile([P, NTILES], i32, name="idx_sb")
            nc.sync.dma_start(idx[:], idx_src)
            zf = gpool.tile([P, NTILES * D2], fp32, name="zf")
            nc.sync.dma_start(zf[:], zf_src)
            zf3 = zf[:].rearrange("p (t d) -> p t d", d=D2)

            out = spool.tile([P, NTILES], fp32, name="out_sb")

            import os as _os
            _noind = bool(_os.environ.get("V2_NO_INDIRECT"))
            mgs = [gpool.tile([P, GT * ROW], fp32, name=f"m_{g}")
                   for g in range(NGROUPS)] if not _ab else []

            def emit_gathers(g):
                mg = mgs[g]
                if _noind:
                    nc.sync.dma_start(
                        mg[:].rearrange("p (t f) -> p t f", f=ROW),
                        table[:].rearrange("(p s) f -> p s f", p=P)[:, 0:GT, :])
                    return
                for t in range(GT):
                    nc.gpsimd.indirect_dma_start(
                        out=mg[:, t * ROW:(t + 1) * ROW],
                        out_offset=None,
                        in_=table[:],
                        in_offset=bass.IndirectOffsetOnAxis(
                            ap=idx[:, g * GT + t:g * GT + t + 1], axis=0),
                    )

            # split each group's tiles between DVE (first GT_D) and GPSIMD
            # (rest): independent buffers, so the two engines run their
            # elimination chains in parallel.  GPSIMD has no reciprocal, so
            # pivots for its part are inverted on DVE each step — emitted
            # INTERLEAVED with the DVE part's steps so the GPSIMD chain is
            # not queued behind the whole DVE chain.
            GT_D = int(_os.environ.get("V2_GTD", "17"))
            parts = [(0, GT_D, nc.vector), (GT_D, GT - GT_D, nc.gpsimd)]
            parts = [(o, c, e) for (o, c, e) in parts if c > 0]

            def emit_elim(g):
                mg = mgs[g]
                views = []
                for (toff, cnt, eng) in parts:
                    m3 = mg.rearrange("p (t f) -> p t f", f=ROW) \
                        [:, toff:toff + cnt, :]
                    mv = m3[:, :, 0:D2 * D2] \
                        .rearrange("p t (i j) -> p t i j", j=D2)
                    dview = m3[:, :, 0:D2 * D2:D2 + 1]      # [P, cnt, 17]
                    nc.vector.tensor_sub(
                        dview, dview,
                        zf3[:, g * GT + toff:g * GT + toff + cnt, :])
                    rg = rpool.tile([P, cnt], fp32, name=f"rg_{g}_{toff}",
                                    tag=f"rg{toff}")
                    csg = rpool.tile([P, cnt * (D2 - 1)], fp32,
                                     name=f"cse_{g}_{toff}", tag=f"cse{toff}")
                    views.append((toff, cnt, eng, m3, mv, dview, rg, csg))
                for j in range(D2 - 1):
                    n = D2 - 1 - j
                    for (toff, cnt, eng, m3, mv, dview, rg, csg) in views:
                        csv = csg.rearrange("p (t i) -> p t i",
                                            i=D2 - 1)[:, :, :n]
                        if eng is nc.vector:
                            nc.vector.reciprocal(rg[:], mv[:, :, j, j])
                            eng.tensor_mul(
                                csv, mv[:, :, j + 1:, j],
                                rg[:].unsqueeze(2).broadcast_to([P, cnt, n]))
                        else:
                            # Q7 divide keeps the GPSIMD chain self-contained
                            # (no per-step wait on the DVE queue)
                            eng.tensor_tensor(
                                csv, mv[:, :, j + 1:, j],
                                mv[:, :, j, j].unsqueeze(2)
                                .broadcast_to([P, cnt, n]),
                                op=OP.divide)
                        tt = d2pool.tile([P, cnt * n * n], fp32,
                                         name=f"te_{g}_{toff}_{j}",
                                         tag=f"tv{toff}")
                        tv4 = tt.rearrange("p (t i j) -> p t i j", i=n, j=n)
                        eng.tensor_mul(
                            tv4,
                            csv.unsqueeze(3).broadcast_to([P, cnt, n, n]),
                            mv[:, :, j:j + 1, j + 1:]
                            .broadcast_to([P, cnt, n, n]))
                        eng.tensor_sub(mv[:, :, j + 1:, j + 1:],
                                       mv[:, :, j + 1:, j + 1:], tv4)
                for (toff, cnt, eng, m3, mv, dview, rg, csg) in views:
                    d2e = d2pool.tile([P, cnt * D2], fp32,
                                      name=f"d2e_{g}_{toff}", tag="d2")
                    nc.scalar.activation(d2e[:], dview, AF.Square)
                    lne = d2pool.tile([P, cnt * D2], fp32,
                                      name=f"lne_{g}_{toff}", tag="ln")
                    nc.scalar.activation(lne[:], d2e[:], AF.Ln)
                    red = rpool.tile([P, cnt], fp32, name=f"red_{g}_{toff}",
                                     tag="red")
                    nc.vector.tensor_reduce(
                        red[:].unsqueeze(2),
                        lne[:].rearrange("p (t d) -> p t d", d=D2),
                        axis=AX.X, op=OP.add)
                    nc.vector.tensor_add(
                        out[:, g * GT + toff:g * GT + toff + cnt], red[:],
                        m3[:, :, D2 * D2])

            if not _ab:
                # stagger gather issue so the Pool engine alternates between
                # descriptor generation and its elimination share
                emit_gathers(0)
                emit_gathers(1)
                for g in range(NGROUPS):
                    emit_elim(g)
                    if g + 2 < NGROUPS:
                        emit_gathers(g + 2)

            if _ab:
                nc.vector.memset(out[:], 0.0)
                nc.vector.tensor_add(out[:, 0:1], out[:, 0:1], bp[:])
            nc.sync.dma_start(out_d[:], out[:])

    return nc


def _get():
    if "nc" not in _cache:
        _cache["nc"] = _build()
    return _cache["nc"]


def _legalize_bir(bir_json: bytes) -> bytes:
    """Walrus allows only ONE embedded sem wait per instruction; split extra
    waits into standalone EventSemaphore instructions."""
    import json as _json
    j = _json.loads(bir_json)
    n_split = 0
    for fn in j.get("functions", []):
        for blk in fn.get("blocks", []):
            out = []
            for inst in blk.get("instructions", []):
                si = inst.get("sync_info") or {}
                waits = si.get("on_wait") or []
                if len(waits) > 1:
                    for wi, w in enumerate(waits[:-1]):
                        out.append({
                            "debug": 0,
                            "engine": inst.get("engine", "Unassigned"),
                            "ins": [], "outs": [],
                            "name": f"{inst.get('name','I')}-w{wi}",
                            "opcode": "EventSemaphore",
                            "sync_info": {"on_wait": [w], "on_update": []},
                        })
                        n_split += 1
                    si = dict(si)
                    si["on_wait"] = [waits[-1]]
                    inst = dict(inst)
                    inst["sync_info"] = si
                out.append(inst)
            blk["instructions"] = out
    if n_split:
        print(f"[legalize] split {n_split} extra sem waits")
    return _json.dumps(j).encode()


_patched = False


def _install_patch():
    global _patched
    if _patched:
        return
    import concourse.bass_utils as bu
    import concourse.bass2jax as b2j
    orig = bu.compile_bir_kernel

    def patched(bir_json, tmpdir, neff_name="file.neff"):
        return orig(_legalize_bir(bir_json), tmpdir, neff_name)

    bu.compile_bir_kernel = patched
    b2j.compile_bir_kernel = patched
    _patched = True


def _preprocess(x, B, C):
    """Sort samples by 15-bit prefix, route to cores by top 3 bits, pad,
    and pack each core's inputs into one int32 blob."""
    x = np.ascontiguousarray(np.asarray(x, dtype=np.int32))
    B = np.asarray(B, dtype=np.float32)
    C = np.asarray(C, dtype=np.float32)
    eye = np.eye(DIM, dtype=np.float32)
    z = (1 - x).astype(np.int64)
    prefix = np.zeros(len(x), dtype=np.int64)
    for k in range(KPRE):
        prefix = (prefix << 1) | z[:, k]
    core = (prefix >> (KPRE - 3)).astype(np.int64)
    row = (prefix & (NNODES - 1)).astype(np.int32)

    blobs, counts = [], []
    for c in range(NCORES):
        sel = np.nonzero(core == c)[0]
        ncs = len(sel)
        assert ncs <= CAP, f"core {c} overflow: {ncs} > {CAP}"
        xc = np.empty((CAP, DIM), dtype=np.int32)
        rc = np.empty(CAP, dtype=np.int32)
        xc[:ncs] = x[sel]
        rc[:ncs] = row[sel]
        if ncs < CAP:
            # pad with a valid sample (or a benign one if the core is empty)
            xc[ncs:] = xc[0] if ncs > 0 else 1
            rc[ncs:] = rc[0] if ncs > 0 else 0
        idx_pt = rc.reshape(NTILES, P).T   # out[p, t] <-> shard row t*P+p
        node = c * P + np.arange(P)
        zb = np.empty((P, BOOT), dtype=np.float32)
        for k in range(BOOT):
            zb[:, k] = (node >> (BOOT - 1 - k)) & 1

        zc = (1 - xc[:, KPRE:DIM]).astype(np.float32)      # [CAP, 17]
        zf = np.ascontiguousarray(
            zc.reshape(NTILES, P, D2).transpose(1, 0, 2)).reshape(P, -1)
        blob = np.empty((BLOB_ROWS, DIM), dtype=np.int32)
        blob[0:ZF_ROWS] = zf.reshape(-1).view(np.int32).reshape(ZF_ROWS, DIM)
        blob[ROW_B:ROW_B + 32] = B.view(np.int32)
        blob[ROW_C:ROW_C + 32] = C.view(np.int32)
        blob[ROW_EYE:ROW_EYE + 32] = eye.view(np.int32)
        blob[ROW_ZB:ROW_ZB + 40] = zb.reshape(-1).view(np.int32).reshape(40, DIM)
        blob[ROW_IDX:ROW_IDX + 272] = \
            np.ascontiguousarray(idx_pt).reshape(-1).reshape(272, DIM)
        blobs.append(blob)
        counts.append(ncs)
    return blobs, counts


def _run(x, B, C, ncores=NCORES, trace=False):
    from concourse.bass_utils import run_bass_kernel_spmd
    _install_patch()

    blobs, counts = _preprocess(x, B, C)
    nc = _get()
    in_maps = [{"blob": blobs[c]} for c in range(ncores)]
    res = run_bass_kernel_spmd(nc, in_maps, core_ids=list(range(ncores)),
                               trace=trace)
    return res, counts


def _reduce(res_results, counts):
    total = 0.0
    for c, r in enumerate(res_results):
        o = r["out"]                       # [P, NTILES]
        ncs = counts[c]
        vals = o.T.reshape(-1)             # slot s = t*P+p -> o[p, t]
        total += vals[:ncs].astype(np.float64).sum()
    return np.float32(0.5 * total / BATCH)


def kernel(x, B, C):
    res, counts = _run(x, B, C)
    return _reduce(res.results, counts)
